# revision 1
# baseline (speedup 1.0000x reference)
"""SAGEConv (mean aggregation) + ReLU on 8 Trainium2 NeuronCores.

Problem: h = relu(mean_agg(x, edges) @ W_l.T + b_l + x @ W_r.T)
  x [8, 55296, 64] f32, 221184 random edges, W [256, 64].

Strategy (dst-sharded, all-batch):
  Core c owns destination nodes [c*6912, (c+1)*6912) for ALL 8 batches.
  x is re-laid host-side as node-major rows of 512 (8 batches x 64 feats),
  cast to bf16, split into lo/hi halves (int16 gather-index limit).
  Per core:
    - dma_gather (GPSIMD mlp library) fetches per-edge source rows (1024B)
      in dst-group order -> edge-major msgs tiles [128 edges/chunk, 512] bf16.
    - Selection matrices S[e, d] = (dstloc[e] == d) built on DVE per chunk;
      TensorE computes aggT[feat, dst] = msgs^T @ S with PSUM accumulation
      over chunks (feat-major aggregation -> no transposes anywhere).
    - Self rows (x of own dsts, batch-pair-swapped on host) flow through the
      same path via plain DMA + identity-S matmuls -> xT in PSUM with
      batch-parity-swapped layout.
    - PSUM->SBUF copies assemble combined lhsT tiles [aggT_b ; xT_b] with
      partition-aligned copies (agg scaled by 1/deg on the way).
    - Phase B: one K=128 bf16 matmul per (128 dsts, batch) against stacked
      [W_l;W_r] (parity-swapped variant for odd batches), relu on DVE/ACT,
      contiguous DMA to the per-core output slice.
  Output: concat core slices -> [8, 55296, 256] f32.
"""

import numpy as np

N_NODES = 55296
F_IN = 64
F_HID = 256
BATCH = 8
NCORE = 8
ND = N_NODES // NCORE          # 6912 dsts per core
GSZ = 256                      # dst group size
NG = ND // GSZ                 # 27 groups per core
SB_G = 3                       # groups per superblock
NSB = NG // SB_G               # 9 superblocks
HALF = N_NODES // 2            # 27648
EW = BATCH * F_IN              # 512 elems per node row

_cache = {}


def _build(schedule, has_bias):
    import concourse.bacc as bacc
    import concourse.tile as tile
    import concourse.mybir as mybir
    from concourse.library_config import mlp

    KA, KB = schedule  # tuples of NG ints: chunk counts per (group, half)
    bf16 = mybir.dt.bfloat16
    f32 = mybir.dt.float32

    sb_cols = []
    for s in range(NSB):
        gs = range(s * SB_G, (s + 1) * SB_G)
        sb_cols.append((sum(KA[g] for g in gs), sum(KB[g] for g in gs)))
    tot_cols = sum(a + b for a, b in sb_cols)
    max_sb_cols = max(a + b for a, b in sb_cols)
    tot_idx = tot_cols * 128
    max_s_live = max(KA[g] + KB[g] for g in range(NG)) + 2

    nc = bacc.Bacc(None, target_bir_lowering=False, debug=False)
    with tile.TileContext(nc) as tc:
        with tc.tile_pool(name="dram", bufs=1, space="DRAM") as dram:
            xab_lo = dram.tile([HALF + 1, EW], bf16, kind="ExternalInput")
            xab_hi = dram.tile([HALF + 1, EW], bf16, kind="ExternalInput")
            xself = dram.tile([ND, EW], bf16, kind="ExternalInput")
            gidx = dram.tile([128, tot_idx // 16], mybir.dt.int16, kind="ExternalInput")
            dstloc = dram.tile([128, tot_cols], f32, kind="ExternalInput")
            selfloc = dram.tile([128, 2], f32, kind="ExternalInput")
            iota_rep = dram.tile([128, GSZ], f32, kind="ExternalInput")
            invdeg_rep = dram.tile([128, ND], f32, kind="ExternalInput")
            w_ev = dram.tile([128, F_HID], bf16, kind="ExternalInput")
            w_od = dram.tile([128, F_HID], bf16, kind="ExternalInput")
            if has_bias:
                bias_rep = dram.tile([128, F_HID], f32, kind="ExternalInput")
            out = dram.tile([BATCH, ND, F_HID], f32, kind="ExternalOutput")

            with (
                tc.tile_pool(name="const", bufs=1) as constp,
                tc.tile_pool(name="msgs", bufs=2) as msgsp,
                tc.tile_pool(name="spool", bufs=max_s_live + 2) as spool,
                tc.tile_pool(name="comb", bufs=2) as combp,
                tc.tile_pool(name="hsb", bufs=4) as hsbp,
                tc.tile_pool(name="aggps", bufs=2, space="PSUM") as aggpsp,
                tc.tile_pool(name="hps", bufs=3, space="PSUM") as hpsp,
            ):
                nc.gpsimd.load_library(mlp)

                gidx_t = constp.tile([128, tot_idx // 16], mybir.dt.int16)
                nc.sync.dma_start(out=gidx_t[:], in_=gidx[:])
                dstloc_t = constp.tile([128, tot_cols], f32)
                nc.sync.dma_start(out=dstloc_t[:], in_=dstloc[:])
                selfloc_t = constp.tile([128, 2], f32)
                nc.sync.dma_start(out=selfloc_t[:], in_=selfloc[:])
                iota_t = constp.tile([128, GSZ], f32)
                nc.sync.dma_start(out=iota_t[:], in_=iota_rep[:])
                invdeg_t = constp.tile([128, ND], f32)
                nc.sync.dma_start(out=invdeg_t[:], in_=invdeg_rep[:])
                w_ev_t = constp.tile([128, F_HID], bf16)
                nc.sync.dma_start(out=w_ev_t[:], in_=w_ev[:])
                w_od_t = constp.tile([128, F_HID], bf16)
                nc.sync.dma_start(out=w_od_t[:], in_=w_od[:])
                if has_bias:
                    bias_t = constp.tile([128, F_HID], f32)
                    nc.sync.dma_start(out=bias_t[:], in_=bias_rep[:])

                col_off = 0
                idx_off = 0
                relu_flip = 0
                for s in range(NSB):
                    acols, bcols = sb_cols[s]
                    ncols = acols + bcols
                    gs = list(range(s * SB_G, (s + 1) * SB_G))
                    m_t = msgsp.tile([128, (max_sb_cols + 2 * SB_G) * EW], bf16,
                                     tag="msgs")
                    m3 = m_t[:].rearrange("p (c e) -> p c e", e=EW)
                    for (xsrc, c0, cn) in ((xab_lo, 0, acols),
                                           (xab_hi, acols, bcols)):
                        if cn == 0:
                            continue
                        nidx = cn * 128
                        nc.gpsimd.dma_gather(
                            out_ap=m3[:, c0:c0 + cn, :],
                            in_ap=xsrc[:],
                            idxs_ap=gidx_t[:, idx_off // 16: (idx_off + nidx) // 16],
                            num_idxs=nidx,
                            num_idxs_reg=nidx,
                            elem_size=EW,
                            single_packet=False,
                        )
                        idx_off += nidx
                    for gl, g in enumerate(gs):
                        sc = ncols + 2 * gl
                        nc.sync.dma_start(
                            out=m3[:, sc:sc + 2, :],
                            in_=xself[g * GSZ:(g + 1) * GSZ, :].rearrange(
                                "(c p) e -> p c e", p=128),
                        )

                    comb = [[combp.tile([128, SB_G * GSZ], bf16,
                                        tag=f"comb{par}{fc}",
                                        name=f"comb{par}{fc}")
                             for fc in range(4)] for par in range(2)]

                    a_off = 0
                    b_off = acols
                    for gl, g in enumerate(gs):
                        cols = ([a_off + i for i in range(KA[g])] +
                                [b_off + i for i in range(KB[g])])
                        a_off += KA[g]
                        b_off += KB[g]
                        nchunk = len(cols)
                        s_tiles = []
                        for cc in cols:
                            s_t = spool.tile([128, GSZ], bf16, tag="sel")
                            nc.vector.tensor_tensor(
                                out=s_t[:],
                                in0=iota_t[:],
                                in1=dstloc_t[:, col_off + cc:col_off + cc + 1]
                                .to_broadcast([128, GSZ]),
                                op=mybir.AluOpType.is_equal,
                            )
                            s_tiles.append(s_t)
                        sself_tiles = []
                        for k in range(2):
                            s_t = spool.tile([128, GSZ], bf16, tag="sel")
                            nc.vector.tensor_tensor(
                                out=s_t[:],
                                in0=iota_t[:],
                                in1=selfloc_t[:, k:k + 1].to_broadcast([128, GSZ]),
                                op=mybir.AluOpType.is_equal,
                            )
                            sself_tiles.append(s_t)

                        dsl = slice(gl * GSZ, (gl + 1) * GSZ)
                        ivd = invdeg_t[:, g * GSZ:(g + 1) * GSZ]
                        for fc in range(4):
                            agg_ps = aggpsp.tile([128, GSZ], f32, tag="agg")
                            for ci, cc in enumerate(cols):
                                nc.tensor.matmul(
                                    out=agg_ps[:],
                                    lhsT=m3[:, cc, fc * 128:(fc + 1) * 128],
                                    rhs=s_tiles[ci][:],
                                    start=(ci == 0),
                                    stop=(ci == nchunk - 1),
                                )
                            xts_ps = aggpsp.tile([128, GSZ], f32, tag="xts")
                            for k in range(2):
                                sc = ncols + 2 * gl + k
                                nc.tensor.matmul(
                                    out=xts_ps[:],
                                    lhsT=m3[:, sc, fc * 128:(fc + 1) * 128],
                                    rhs=sself_tiles[k][:],
                                    start=(k == 0),
                                    stop=(k == 1),
                                )
                            # even batch 2fc: agg parts 0:64, x parts 64:128
                            nc.vector.tensor_mul(
                                out=comb[0][fc][:64, dsl],
                                in0=agg_ps[:64, :], in1=ivd[:64, :])
                            nc.scalar.activation(
                                out=comb[0][fc][64:128, dsl],
                                in_=xts_ps[64:128, :],
                                func=mybir.ActivationFunctionType.Copy)
                            # odd batch 2fc+1: x parts 0:64, agg parts 64:128
                            nc.scalar.activation(
                                out=comb[1][fc][:64, dsl],
                                in_=xts_ps[:64, :],
                                func=mybir.ActivationFunctionType.Copy)
                            nc.vector.tensor_mul(
                                out=comb[1][fc][64:128, dsl],
                                in0=agg_ps[64:128, :], in1=ivd[64:128, :])
                    col_off += ncols

                    for b in range(BATCH):
                        fc, par = b // 2, b % 2
                        w_t = w_od_t if par else w_ev_t
                        for dch in range(SB_G * GSZ // 128):
                            h_ps = hpsp.tile([128, F_HID], f32, tag="hps")
                            nc.tensor.matmul(
                                out=h_ps[:],
                                lhsT=comb[par][fc][:, dch * 128:(dch + 1) * 128],
                                rhs=w_t[:],
                                start=True,
                                stop=True,
                            )
                            if has_bias:
                                nc.vector.tensor_add(
                                    out=h_ps[:], in0=h_ps[:], in1=bias_t[:])
                            h_t = hsbp.tile([128, F_HID], f32, tag="hsb")
                            if relu_flip % 3 == 0:
                                nc.scalar.activation(
                                    out=h_t[:], in_=h_ps[:],
                                    func=mybir.ActivationFunctionType.Relu)
                            else:
                                nc.vector.tensor_relu(out=h_t[:], in_=h_ps[:])
                            relu_flip += 1
                            r0 = s * SB_G * GSZ + dch * 128
                            nc.sync.dma_start(
                                out=out[b, r0:r0 + 128, :], in_=h_t[:])
    nc.compile()
    names = dict(
        xab_lo=xab_lo.name, xab_hi=xab_hi.name, xself=xself.name,
        gidx=gidx.name, dstloc=dstloc.name, selfloc=selfloc.name,
        iota_rep=iota_rep.name, invdeg_rep=invdeg_rep.name,
        w_ev=w_ev.name, w_od=w_od.name, out=out.name,
        bias_rep=(bias_rep.name if has_bias else None),
    )
    return nc, names


def _prep(x, edge_src, edge_dst, W_l, b_l, W_r):
    from ml_dtypes import bfloat16

    deg = np.bincount(edge_dst, minlength=N_NODES).astype(np.float32)
    invdeg = (1.0 / np.maximum(deg, 1.0)).astype(np.float32)

    xn = np.ascontiguousarray(x.transpose(1, 0, 2)).reshape(N_NODES, EW)
    xn_bf = xn.astype(bfloat16)
    zrow = np.zeros((1, EW), dtype=bfloat16)
    xab_lo = np.ascontiguousarray(np.vstack([xn_bf[:HALF], zrow]))
    xab_hi = np.ascontiguousarray(np.vstack([xn_bf[HALF:], zrow]))

    # batch-pair swapped feature order for the self rows
    swap = np.arange(EW).reshape(BATCH, F_IN)
    swap = swap.reshape(4, 2, F_IN)[:, ::-1, :].reshape(EW)

    core = edge_dst // ND
    per_core = []
    counts = np.zeros((NCORE, NG, 2), np.int64)
    for c in range(NCORE):
        sel = core == c
        ed = (edge_dst[sel] - c * ND).astype(np.int64)
        es = edge_src[sel].astype(np.int64)
        g = ed // GSZ
        h = (es >= HALF).astype(np.int64)
        order = np.lexsort((es, h, g))
        ed, es, g, h = ed[order], es[order], g[order], h[order]
        key = g * 2 + h
        bounds = np.searchsorted(key, np.arange(2 * NG + 1))
        cnt = np.diff(bounds).reshape(NG, 2)
        counts[c] = cnt
        per_core.append((ed, es, bounds))

    K = np.ceil(counts.max(axis=0) / 128).astype(np.int64)
    K = np.maximum(K, 1)
    KA = tuple(int(v) for v in K[:, 0])
    KB = tuple(int(v) for v in K[:, 1])

    # canonical column order: per sb, A cols of its groups then B cols
    col_group = []
    for s in range(NSB):
        gs = range(s * SB_G, (s + 1) * SB_G)
        for g in gs:
            col_group += [(g, 0)] * KA[g]
        for g in gs:
            col_group += [(g, 1)] * KB[g]
    tot_cols = len(col_group)
    gh_cols = {}
    for ci, gh in enumerate(col_group):
        gh_cols.setdefault(gh, []).append(ci)

    iota_rep = np.broadcast_to(
        np.arange(GSZ, dtype=np.float32)[None, :], (128, GSZ)).copy()
    selfloc = np.stack([np.arange(128, dtype=np.float32),
                        np.arange(128, 256, dtype=np.float32)], axis=1).copy()

    WlT = W_l.T.astype(np.float32)
    WrT = W_r.T.astype(np.float32)
    w_ev = np.vstack([WlT, WrT]).astype(bfloat16)
    w_od = np.vstack([WrT, WlT]).astype(bfloat16)
    has_bias = bool(np.any(b_l != 0))
    bias_rep = (np.broadcast_to(b_l.astype(np.float32)[None, :],
                                (128, F_HID)).copy() if has_bias else None)

    in_maps = []
    for c in range(NCORE):
        ed, es, bounds = per_core[c]
        slotvals = np.full((tot_cols, 128), HALF, dtype=np.int16)
        dl = np.full((tot_cols, 128), -1.0, dtype=np.float32)
        for gg in range(NG):
            for hh in range(2):
                lo, hi = bounds[2 * gg + hh], bounds[2 * gg + hh + 1]
                cnt = hi - lo
                cols = gh_cols[(gg, hh)]
                buf = np.full(len(cols) * 128, HALF, np.int16)
                dbuf = np.full(len(cols) * 128, -1.0, np.float32)
                if cnt:
                    buf[:cnt] = (es[lo:hi] - (HALF if hh else 0)).astype(np.int16)
                    dbuf[:cnt] = (ed[lo:hi] - gg * GSZ).astype(np.float32)
                for j, ci in enumerate(cols):
                    slotvals[ci] = buf[j * 128:(j + 1) * 128]
                    dl[ci] = dbuf[j * 128:(j + 1) * 128]
        chunks = []
        ci = 0
        for s in range(NSB):
            gs = range(s * SB_G, (s + 1) * SB_G)
            na = sum(KA[g2] for g2 in gs)
            nb = sum(KB[g2] for g2 in gs)
            for cn in (na, nb):
                if cn:
                    sl = slotvals[ci:ci + cn].reshape(-1)
                    chunks.append(np.tile(sl.reshape(-1, 16).T, (8, 1)))
                    ci += cn
        gidx_arr = np.ascontiguousarray(np.concatenate(chunks, axis=1))

        invdeg_c = np.broadcast_to(
            invdeg[c * ND:(c + 1) * ND][None, :], (128, ND)).copy()
        xself_c = np.ascontiguousarray(xn_bf[c * ND:(c + 1) * ND][:, swap])

        in_maps.append(dict(
            xab_lo=xab_lo, xab_hi=xab_hi, xself=xself_c,
            gidx=gidx_arr, dstloc=np.ascontiguousarray(dl.T),
            selfloc=selfloc, iota_rep=iota_rep, invdeg_rep=invdeg_c,
            w_ev=w_ev, w_od=w_od, bias_rep=bias_rep,
        ))
    return (KA, KB), has_bias, in_maps


def kernel(x, edge_src, edge_dst, W_l, b_l, W_r):
    from concourse.bass_utils import run_bass_kernel_spmd

    x = np.asarray(x, dtype=np.float32)
    edge_src = np.asarray(edge_src, dtype=np.int32)
    edge_dst = np.asarray(edge_dst, dtype=np.int32)
    W_l = np.asarray(W_l, dtype=np.float32)
    b_l = np.asarray(b_l, dtype=np.float32)
    W_r = np.asarray(W_r, dtype=np.float32)

    schedule, has_bias, in_maps = _prep(x, edge_src, edge_dst, W_l, b_l, W_r)
    key = (schedule, has_bias)
    if key not in _cache:
        _cache[key] = _build(schedule, has_bias)
    nc, names = _cache[key]

    run_maps = []
    for m in in_maps:
        rm = {names[k]: v for k, v in m.items()
              if names.get(k) is not None and v is not None}
        run_maps.append(rm)
    res = run_bass_kernel_spmd(nc, run_maps, list(range(NCORE)))
    outs = [res.results[c][names["out"]] for c in range(NCORE)]
    return np.concatenate(outs, axis=1)



# revision 10
# speedup vs baseline: 1.9137x; 1.9137x over previous
"""SAGEConv (mean aggregation) + ReLU on 8 Trainium2 NeuronCores.

Problem: h = relu(mean_agg(x, edges) @ W_l.T + b_l + x @ W_r.T)
  x [8, 55296, 64] f32, 221184 random edges, W [256, 64].

Strategy (dst-sharded, all-batch, v2 — descgen-optimized):
  Core c owns destination nodes [c*6912, (c+1)*6912) for ALL 8 batches.
  x re-laid host-side as node-major rows of 512 (8 batches x 64 feats) bf16,
  split into lo/hi halves (int16 gather-index limit).
  Per core, per superblock (768 dsts = 6 groups of 128):
    - One dma_gather per (sb, half) with edges PACKED (sorted by dst) and
      trailing -1 padding: the Q7 descgen trims trailing negatives per-core,
      so the serial GPSIMD descriptor generation (~9.6ns/row, the pipeline
      pole) covers only real edges.
    - Selection matrices S[e, d] = (dstloc[e] == d) * invdeg[d] built on DVE
      (fp16 compare + fp16 scale -> bf16), one strip per (half, group) run of
      chunks. Chunks crossing a group boundary matmul into both groups.
    - TensorE accumulates aggT[feat128, 4fc x 128dst] per group into a full
      PSUM bank; scaled agg is copied (Scalar engine) into packed comb tiles
      [agg64 ; x64] per batch-parity; the x half arrives by direct DMA from a
      host-transposed xselfT (no identity matmuls).
    - Phase B: one K=128 bf16 matmul per (128 dsts, batch) against stacked
      [W_l;W_r] (parity-swapped for odd batches), relu merged over 2 dst
      chunks ([128,512] PSUM), bf16 output DMA (one per 256 dsts).
  Output: bf16 [8, 6912, 256] per core -> host concat + upcast to f32.
"""

import os
import numpy as np

_NQUEUES = int(os.environ.get("K_NQUEUES", "4"))
_NEGPAD = int(os.environ.get("K_NEGPAD", "0"))    # 0: pad all calls w/ zero row
_SBUILD = os.environ.get("K_SBUILD", "strip")      # strip | simple

N_NODES = 55296
F_IN = 64
F_HID = 256
BATCH = 8
NCORE = 8
ND = N_NODES // NCORE          # 6912 dsts per core
GSZ = 128                      # dst group size (PSUM bank: 4fc x 128 dsts)
NGL = 6                        # groups per superblock
SBD = GSZ * NGL                # 768 dsts per superblock
NSB = ND // SBD                # 9 superblocks
HALF = N_NODES // 2            # 27648
EW = BATCH * F_IN              # 512 elems per node row
MSGS_BUFS = 2                  # msgs pool depth (first MSGS_BUFS sbs pad with
                               # the zero row instead of -1: no stale SBUF)
SRUN = 6                       # max chunks per S-build strip

_cache = {}


def _build(schedule, has_bias):
    import concourse.bacc as bacc
    import concourse.tile as tile
    import concourse.mybir as mybir
    from concourse.library_config import mlp

    K, targets = schedule  # K[sb][h] chunk counts; targets[sb][h][k]=(gmin,gmax)
    bf16 = mybir.dt.bfloat16
    fp16 = mybir.dt.float16
    f32 = mybir.dt.float32

    sb_cols = [K[s][0] + K[s][1] for s in range(NSB)]
    tot_cols = sum(sb_cols)
    max_sb_cols = max(sb_cols)
    tot_idx = tot_cols * 128

    nc = bacc.Bacc(None, target_bir_lowering=False, debug=False,
                   num_swdge_queues=_NQUEUES)
    with tile.TileContext(nc) as tc:
        with tc.tile_pool(name="dram", bufs=1, space="DRAM") as dram:
            xab_lo = dram.tile([HALF + 1, EW], bf16, kind="ExternalInput")
            xab_hi = dram.tile([HALF + 1, EW], bf16, kind="ExternalInput")
            xselfT = dram.tile([BATCH, F_IN, ND], bf16, kind="ExternalInput")
            gidx = dram.tile([128, tot_idx // 16], mybir.dt.int16,
                             kind="ExternalInput")
            dstloc = dram.tile([128, tot_cols], fp16, kind="ExternalInput")
            iota_rep = dram.tile([128, SBD], fp16, kind="ExternalInput")
            invdeg_rep = dram.tile([128, ND], fp16, kind="ExternalInput")
            w_ev = dram.tile([128, F_HID], bf16, kind="ExternalInput")
            w_od = dram.tile([128, F_HID], bf16, kind="ExternalInput")
            if has_bias:
                bias_rep = dram.tile([128, 2 * F_HID], f32,
                                     kind="ExternalInput")
            out = dram.tile([BATCH, ND, F_HID], bf16, kind="ExternalOutput")

            with (
                tc.tile_pool(name="const", bufs=1) as constp,
                tc.tile_pool(name="msgs", bufs=MSGS_BUFS) as msgsp,
                tc.tile_pool(name="seq", bufs=4) as seqp,
                tc.tile_pool(name="sel", bufs=26) as selp,
                tc.tile_pool(name="comb", bufs=4) as combp,
                tc.tile_pool(name="hsb", bufs=4) as hsbp,
                tc.tile_pool(name="aggps", bufs=3, space="PSUM") as aggpsp,
                tc.tile_pool(name="hps", bufs=3, space="PSUM") as hpsp,
            ):
                nc.gpsimd.load_library(mlp)

                gidx_t = constp.tile([128, tot_idx // 16], mybir.dt.int16)
                nc.sync.dma_start(out=gidx_t[:], in_=gidx[:])
                dstloc_t = constp.tile([128, tot_cols], fp16)
                nc.sync.dma_start(out=dstloc_t[:], in_=dstloc[:])
                iota_t = constp.tile([128, SBD], fp16)
                nc.sync.dma_start(out=iota_t[:], in_=iota_rep[:])
                invdeg_t = constp.tile([128, ND], fp16)
                nc.sync.dma_start(out=invdeg_t[:], in_=invdeg_rep[:])
                w_ev_t = constp.tile([128, F_HID], bf16)
                nc.sync.dma_start(out=w_ev_t[:], in_=w_ev[:])
                w_od_t = constp.tile([128, F_HID], bf16)
                nc.sync.dma_start(out=w_od_t[:], in_=w_od[:])
                if has_bias:
                    bias_t = constp.tile([128, 2 * F_HID], f32)
                    nc.sync.dma_start(out=bias_t[:], in_=bias_rep[:])

                col_off = 0
                idx_off = 0
                relu_flip = 0
                for s in range(NSB):
                    KA, KB = K[s]
                    ncols = KA + KB
                    m_t = msgsp.tile([128, max_sb_cols * EW], bf16,
                                     tag="msgs")
                    m3 = m_t[:].rearrange("p (c e) -> p c e", e=EW)
                    for (xsrc, c0, cn) in ((xab_lo, 0, KA),
                                           (xab_hi, KA, KB)):
                        if cn == 0:
                            continue
                        nidx = cn * 128
                        nc.gpsimd.dma_gather(
                            out_ap=m3[:, c0:c0 + cn, :],
                            in_ap=xsrc[:],
                            idxs_ap=gidx_t[:, idx_off // 16:
                                           (idx_off + nidx) // 16],
                            num_idxs=nidx,
                            num_idxs_reg=nidx,
                            elem_size=EW,
                            single_packet=False,
                            queue_num=(2 * s + (0 if c0 == 0 else 1))
                            % _NQUEUES,
                        )
                        idx_off += nidx

                    comb = [combp.tile([128, 4 * SBD], bf16, tag=f"comb{par}",
                                       name=f"comb{par}_{s}")
                            for par in range(2)]
                    # x/self half of comb straight from DRAM (transposed host
                    # layout); even batch -> partitions 64:128 of comb[0],
                    # odd batch -> partitions 0:64 of comb[1]
                    for b in range(BATCH):
                        par, fc = b % 2, b // 2
                        p0 = 64 if par == 0 else 0
                        nc.sync.dma_start(
                            out=comb[par][p0:p0 + 64,
                                          fc * SBD:(fc + 1) * SBD],
                            in_=xselfT[b, :, s * SBD:(s + 1) * SBD],
                        )

                    # per-group chunk lists from shared targets
                    tg = list(targets[s][0]) + list(targets[s][1])
                    g_cols = [[ci for ci in range(ncols)
                               if tg[ci][0] <= g <= tg[ci][1]]
                              for g in range(NGL)]
                    first_col = [cols[0] for cols in g_cols]
                    last_col = [cols[-1] for cols in g_cols]

                    # S strips per (half, group): contiguous runs of chunks,
                    # fp16 equality * fp16 invdeg -> bf16, <=SRUN cols per op
                    s_ap = {}   # (g, ci) -> rhs AP [128, 128]
                    for h, (lo, hi) in ((0, (0, KA)), (1, (KA, ncols))):
                        for g in range(NGL):
                            cols = [ci for ci in g_cols[g] if lo <= ci < hi]
                            if not cols:
                                continue
                            assert cols == list(range(cols[0], cols[-1] + 1))
                            if _SBUILD == "simple":
                                for ci in cols:
                                    eq_t = seqp.tile([128, SRUN * 128], fp16,
                                                     tag="seq",
                                                     name=f"eq_{s}_{h}_{g}_{ci}")
                                    nc.vector.tensor_tensor(
                                        out=eq_t[:, 0:128],
                                        in0=iota_t[:, g * 128:(g + 1) * 128],
                                        in1=dstloc_t[:, col_off + ci:
                                                     col_off + ci + 1]
                                        .to_broadcast([128, 128]),
                                        op=mybir.AluOpType.is_equal,
                                    )
                                    s_t = selp.tile([128, SRUN * 128], bf16,
                                                    tag="sel",
                                                    name=f"s_{s}_{h}_{g}_{ci}")
                                    nc.vector.tensor_tensor(
                                        out=s_t[:, 0:128],
                                        in0=eq_t[:, 0:128],
                                        in1=invdeg_t[:, s * SBD + g * 128:
                                                     s * SBD + (g + 1) * 128],
                                        op=mybir.AluOpType.mult,
                                    )
                                    s_ap[(g, ci)] = s_t[:, 0:128]
                                continue
                            for a in range(0, len(cols), SRUN):
                                run = cols[a:a + SRUN]
                                n = len(run)
                                c0 = run[0]
                                eq_t = seqp.tile([128, SRUN * 128], fp16,
                                                 tag="seq")
                                eq3 = eq_t[:].rearrange("p (c d) -> p c d",
                                                        d=128)
                                nc.vector.tensor_tensor(
                                    out=eq3[:, 0:n, :],
                                    in0=iota_t[:, g * 128:(g + 1) * 128]
                                    .rearrange("p (o d) -> p o d", o=1)
                                    .to_broadcast([128, n, 128]),
                                    in1=dstloc_t[:, col_off + c0:
                                                 col_off + c0 + n]
                                    .rearrange("p (c o) -> p c o", o=1)
                                    .to_broadcast([128, n, 128]),
                                    op=mybir.AluOpType.is_equal,
                                )
                                s_t = selp.tile([128, SRUN * 128], bf16,
                                                tag="sel")
                                s3 = s_t[:].rearrange("p (c d) -> p c d",
                                                      d=128)
                                nc.vector.tensor_tensor(
                                    out=s3[:, 0:n, :],
                                    in0=eq3[:, 0:n, :],
                                    in1=invdeg_t[:, s * SBD + g * 128:
                                                 s * SBD + (g + 1) * 128]
                                    .rearrange("p (o d) -> p o d", o=1)
                                    .to_broadcast([128, n, 128]),
                                    op=mybir.AluOpType.mult,
                                )
                                for j, ci in enumerate(run):
                                    s_ap[(g, ci)] = s3[:, j, :]

                    # aggregation matmuls, PSUM bank per group. start=True
                    # clears the has-written bits of the WHOLE bank (and
                    # accumulate-mode overwrites elements with cleared bits),
                    # so exactly ONE start per bank; everything else
                    # accumulates.
                    agg = {}
                    for ci in range(ncols):
                        gmin, gmax = tg[ci]
                        for g in range(gmin, gmax + 1):
                            fresh = g not in agg
                            if fresh:
                                agg[g] = aggpsp.tile([128, 512], f32,
                                                     tag="agg",
                                                     name=f"agg_{s}_{g}")
                            for fc in range(4):
                                nc.tensor.matmul(
                                    out=agg[g][:, fc * 128:(fc + 1) * 128],
                                    lhsT=m3[:, ci, fc * 128:(fc + 1) * 128],
                                    rhs=s_ap[(g, ci)],
                                    start=(fresh and fc == 0),
                                    stop=(ci == last_col[g] and fc == 3),
                                    skip_group_check=True,
                                )
                        # groups completed by this chunk -> copy into comb
                        for g in range(gmin, gmax + 1):
                            if last_col[g] != ci:
                                continue
                            a4 = agg[g][:].rearrange("p (f d) -> p f d", f=4)
                            c4 = [comb[par][:].rearrange("p (f d) -> p f d",
                                                         f=4)
                                  for par in range(2)]
                            dsl = slice(g * 128, (g + 1) * 128)
                            # even batches: agg parts 0:64 of comb[0]
                            nc.scalar.activation(
                                out=c4[0][0:64, :, dsl],
                                in_=a4[0:64, :, :],
                                func=mybir.ActivationFunctionType.Copy)
                            # odd batches: agg parts 64:128 of comb[1]
                            nc.scalar.activation(
                                out=c4[1][64:128, :, dsl],
                                in_=a4[64:128, :, :],
                                func=mybir.ActivationFunctionType.Copy)

                    # phase B: h = [agg;x] @ [W_l;W_r], relu, bf16 out
                    for b in range(BATCH):
                        par, fc = b % 2, b // 2
                        w_t = w_od_t if par else w_ev_t
                        for d2 in range(SBD // 256):
                            h_ps = hpsp.tile([128, 512], f32, tag="hps")
                            for j in range(2):
                                dch = d2 * 2 + j
                                nc.tensor.matmul(
                                    out=h_ps[:, j * 256:(j + 1) * 256],
                                    lhsT=comb[par][:, fc * SBD + dch * 128:
                                                   fc * SBD + (dch + 1) * 128],
                                    rhs=w_t[:],
                                    start=True,
                                    stop=True,
                                )
                            if has_bias:
                                nc.vector.tensor_add(
                                    out=h_ps[:], in0=h_ps[:], in1=bias_t[:])
                            h_t = hsbp.tile([128, 512], bf16, tag="hsb")
                            if relu_flip % 2 == 0:
                                nc.scalar.activation(
                                    out=h_t[:], in_=h_ps[:],
                                    func=mybir.ActivationFunctionType.Relu)
                            else:
                                nc.vector.tensor_relu(out=h_t[:], in_=h_ps[:])
                            relu_flip += 1
                            r0 = s * SBD + d2 * 256
                            nc.sync.dma_start(
                                out=out[b, r0:r0 + 256, :]
                                .rearrange("(k p) h -> p k h", p=128),
                                in_=h_t[:].rearrange("p (k h) -> p k h", k=2),
                            )
                    col_off += ncols
    nc.compile()
    names = dict(
        xab_lo=xab_lo.name, xab_hi=xab_hi.name, xselfT=xselfT.name,
        gidx=gidx.name, dstloc=dstloc.name, iota_rep=iota_rep.name,
        invdeg_rep=invdeg_rep.name, w_ev=w_ev.name, w_od=w_od.name,
        out=out.name, bias_rep=(bias_rep.name if has_bias else None),
    )
    return nc, names


def _prep(x, edge_src, edge_dst, W_l, b_l, W_r):
    from ml_dtypes import bfloat16

    deg = np.bincount(edge_dst, minlength=N_NODES).astype(np.float32)
    invdeg = (1.0 / np.maximum(deg, 1.0)).astype(np.float16)

    xn = np.ascontiguousarray(x.transpose(1, 0, 2)).reshape(N_NODES, EW)
    xn_bf = xn.astype(bfloat16)
    zrow = np.zeros((1, EW), dtype=bfloat16)
    xab_lo = np.ascontiguousarray(np.vstack([xn_bf[:HALF], zrow]))
    xab_hi = np.ascontiguousarray(np.vstack([xn_bf[HALF:], zrow]))

    core = edge_dst // ND
    per_core = []
    cnt = np.zeros((NCORE, NSB, 2), np.int64)
    for c in range(NCORE):
        sel = core == c
        ed = (edge_dst[sel] - c * ND).astype(np.int64)
        es = edge_src[sel].astype(np.int64)
        sb = ed // SBD
        h = (es >= HALF).astype(np.int64)
        order = np.lexsort((es, ed, h, sb))
        ed, es, sb, h = ed[order], es[order], sb[order], h[order]
        key = sb * 2 + h
        bounds = np.searchsorted(key, np.arange(2 * NSB + 1))
        cnt[c] = np.diff(bounds).reshape(NSB, 2)
        per_core.append((ed, es, bounds))

    Kmat = np.ceil(cnt.max(axis=0) / 128).astype(np.int64)
    Kmat = np.maximum(Kmat, 1)
    K = tuple((int(Kmat[s, 0]), int(Kmat[s, 1])) for s in range(NSB))

    # shared chunk -> group-range targets (union over cores)
    gmin = np.full((NSB, 2, int(Kmat.max())), NGL, np.int64)
    gmax = np.full((NSB, 2, int(Kmat.max())), -1, np.int64)
    for c in range(NCORE):
        ed, es, bounds = per_core[c]
        for s in range(NSB):
            for h in range(2):
                lo, hi = bounds[2 * s + h], bounds[2 * s + h + 1]
                n = hi - lo
                gl = (ed[lo:hi] - s * SBD) // GSZ
                for k in range(int(Kmat[s, h])):
                    a, b = k * 128, min((k + 1) * 128, n)
                    if a >= n:
                        break
                    gmin[s, h, k] = min(gmin[s, h, k], gl[a])
                    gmax[s, h, k] = max(gmax[s, h, k], gl[b - 1])
    # chunks with no real edges on any core: harmless all-zero target
    for s in range(NSB):
        for h in range(2):
            for k in range(int(Kmat[s, h])):
                if gmax[s, h, k] < 0:
                    gmin[s, h, k] = 0
                    gmax[s, h, k] = 0
        # ensure every group is targeted at least once (start=True zeroing)
        covered = np.zeros(NGL, bool)
        for h in range(2):
            for k in range(int(Kmat[s, h])):
                covered[gmin[s, h, k]:gmax[s, h, k] + 1] = True
        for g in range(NGL):
            if not covered[g]:
                h = 0 if Kmat[s, 0] > 0 else 1
                gmin[s, h, 0] = min(gmin[s, h, 0], g)
                gmax[s, h, 0] = max(gmax[s, h, 0], g)
                covered[gmin[s, h, 0]:gmax[s, h, 0] + 1] = True
    targets = tuple(
        tuple(tuple((int(gmin[s, h, k]), int(gmax[s, h, k]))
                    for k in range(int(Kmat[s, h])))
              for h in range(2))
        for s in range(NSB))
    schedule = (K, targets)

    iota_rep = np.broadcast_to(
        np.arange(SBD, dtype=np.float16)[None, :], (128, SBD)).copy()

    WlT = W_l.T.astype(np.float32)
    WrT = W_r.T.astype(np.float32)
    w_ev = np.vstack([WlT, WrT]).astype(bfloat16)
    w_od = np.vstack([WrT, WlT]).astype(bfloat16)
    has_bias = bool(np.any(b_l != 0))
    bias_rep = (np.broadcast_to(
        np.tile(b_l.astype(np.float32), 2)[None, :],
        (128, 2 * F_HID)).copy() if has_bias else None)

    in_maps = []
    for c in range(NCORE):
        ed, es, bounds = per_core[c]
        gidx_chunks = []
        dl_cols = []
        for s in range(NSB):
            for h in range(2):
                kk = int(Kmat[s, h])
                if kk == 0:
                    continue
                lo, hi = bounds[2 * s + h], bounds[2 * s + h + 1]
                n = hi - lo
                pad = HALF if (s < MSGS_BUFS or not _NEGPAD) else -1
                buf = np.full(kk * 128, pad, np.int16)
                buf[:n] = (es[lo:hi] - h * HALF).astype(np.int16)
                dbuf = np.full(kk * 128, -1.0, np.float16)
                dbuf[:n] = (ed[lo:hi] - s * SBD).astype(np.float16)
                gidx_chunks.append(np.tile(buf.reshape(-1, 16).T, (8, 1)))
                dl_cols.append(dbuf.reshape(kk, 128))
        gidx_arr = np.ascontiguousarray(
            np.concatenate(gidx_chunks, axis=1))
        dl = np.concatenate(dl_cols, axis=0)            # [tot_cols, 128]
        dstloc_arr = np.ascontiguousarray(dl.T)          # [128, tot_cols]

        xselfT_c = np.ascontiguousarray(
            xn_bf[c * ND:(c + 1) * ND].T).reshape(BATCH, F_IN, ND)
        invdeg_c = np.broadcast_to(
            invdeg[c * ND:(c + 1) * ND][None, :], (128, ND)).copy()

        in_maps.append(dict(
            xab_lo=xab_lo, xab_hi=xab_hi, xselfT=xselfT_c,
            gidx=gidx_arr, dstloc=dstloc_arr, iota_rep=iota_rep,
            invdeg_rep=invdeg_c, w_ev=w_ev, w_od=w_od, bias_rep=bias_rep,
        ))
    return schedule, has_bias, in_maps


def kernel(x, edge_src, edge_dst, W_l, b_l, W_r):
    from concourse.bass_utils import run_bass_kernel_spmd

    x = np.asarray(x, dtype=np.float32)
    edge_src = np.asarray(edge_src, dtype=np.int32)
    edge_dst = np.asarray(edge_dst, dtype=np.int32)
    W_l = np.asarray(W_l, dtype=np.float32)
    b_l = np.asarray(b_l, dtype=np.float32)
    W_r = np.asarray(W_r, dtype=np.float32)

    schedule, has_bias, in_maps = _prep(x, edge_src, edge_dst, W_l, b_l, W_r)
    key = (schedule, has_bias)
    if key not in _cache:
        _cache[key] = _build(schedule, has_bias)
    nc, names = _cache[key]

    run_maps = []
    for m in in_maps:
        rm = {names[k]: v for k, v in m.items()
              if names.get(k) is not None and v is not None}
        run_maps.append(rm)
    res = run_bass_kernel_spmd(nc, run_maps, list(range(NCORE)))
    outs = [np.asarray(res.results[c][names["out"]]) for c in range(NCORE)]
    return np.concatenate(outs, axis=1).astype(np.float32)


# revision 12
# speedup vs baseline: 1.9658x; 1.0272x over previous
"""SAGEConv (mean aggregation) + ReLU on 8 Trainium2 NeuronCores.

Problem: h = relu(mean_agg(x, edges) @ W_l.T + b_l + x @ W_r.T)
  x [8, 55296, 64] f32, 221184 random edges, W [256, 64].

Strategy (dst-sharded, all-batch, v2 — descgen-optimized):
  Core c owns destination nodes [c*6912, (c+1)*6912) for ALL 8 batches.
  x re-laid host-side as node-major rows of 512 (8 batches x 64 feats) bf16,
  split into lo/hi halves (int16 gather-index limit).
  Per core, per superblock (768 dsts = 6 groups of 128):
    - One dma_gather per (sb, half) with edges PACKED (sorted by dst) and
      trailing -1 padding: the Q7 descgen trims trailing negatives per-core,
      so the serial GPSIMD descriptor generation (~9.6ns/row, the pipeline
      pole) covers only real edges.
    - Selection matrices S[e, d] = (dstloc[e] == d) * invdeg[d] built on DVE
      (fp16 compare + fp16 scale -> bf16), one strip per (half, group) run of
      chunks. Chunks crossing a group boundary matmul into both groups.
    - TensorE accumulates aggT[feat128, 4fc x 128dst] per group into a full
      PSUM bank; scaled agg is copied (Scalar engine) into packed comb tiles
      [agg64 ; x64] per batch-parity; the x half arrives by direct DMA from a
      host-transposed xselfT (no identity matmuls).
    - Phase B: one K=128 bf16 matmul per (128 dsts, batch) against stacked
      [W_l;W_r] (parity-swapped for odd batches), relu merged over 2 dst
      chunks ([128,512] PSUM), bf16 output DMA (one per 256 dsts).
  Output: bf16 [8, 6912, 256] per core -> host concat + upcast to f32.
"""

import os
import numpy as np

_NQUEUES = int(os.environ.get("K_NQUEUES", "4"))
_NEGPAD = int(os.environ.get("K_NEGPAD", "0"))    # 0: pad all calls w/ zero row
_SBUILD = os.environ.get("K_SBUILD", "strip")      # strip | simple

N_NODES = 55296
F_IN = 64
F_HID = 256
BATCH = 8
NCORE = 8
ND = N_NODES // NCORE          # 6912 dsts per core
GSZ = 128                      # dst group size (PSUM bank: 4fc x 128 dsts)
NGL = 6                        # groups per superblock
SBD = GSZ * NGL                # 768 dsts per superblock
NSB = ND // SBD                # 9 superblocks
HALF = N_NODES // 2            # 27648
EW = BATCH * F_IN              # 512 elems per node row
MSGS_BUFS = 3                  # msgs pool depth (first MSGS_BUFS sbs pad with
                               # the zero row instead of -1: no stale SBUF)
SRUN = 6                       # max chunks per S-build strip

_cache = {}


def _build(schedule, has_bias):
    import concourse.bacc as bacc
    import concourse.tile as tile
    import concourse.mybir as mybir
    from concourse.library_config import mlp

    K, targets = schedule  # K[sb][h] chunk counts; targets[sb][h][k]=(gmin,gmax)
    bf16 = mybir.dt.bfloat16
    fp16 = mybir.dt.float16
    f32 = mybir.dt.float32

    sb_cols = [K[s][0] + K[s][1] for s in range(NSB)]
    tot_cols = sum(sb_cols)
    max_sb_cols = max(sb_cols)
    tot_idx = tot_cols * 128

    nc = bacc.Bacc(None, target_bir_lowering=False, debug=False,
                   num_swdge_queues=_NQUEUES)
    with tile.TileContext(nc) as tc:
        with tc.tile_pool(name="dram", bufs=1, space="DRAM") as dram:
            xab_lo = dram.tile([HALF + 1, EW], bf16, kind="ExternalInput")
            xab_hi = dram.tile([HALF + 1, EW], bf16, kind="ExternalInput")
            xselfT = dram.tile([BATCH, F_IN, ND], bf16, kind="ExternalInput")
            gidx = dram.tile([128, tot_idx // 16], mybir.dt.int16,
                             kind="ExternalInput")
            dstloc = dram.tile([128, tot_cols], fp16, kind="ExternalInput")
            iota_rep = dram.tile([128, SBD], fp16, kind="ExternalInput")
            invdeg_rep = dram.tile([128, ND], fp16, kind="ExternalInput")
            w_ev = dram.tile([128, F_HID], bf16, kind="ExternalInput")
            w_od = dram.tile([128, F_HID], bf16, kind="ExternalInput")
            if has_bias:
                bias_rep = dram.tile([128, 2 * F_HID], f32,
                                     kind="ExternalInput")
            out = dram.tile([BATCH, ND, F_HID], bf16, kind="ExternalOutput")

            with (
                tc.tile_pool(name="const", bufs=1) as constp,
                tc.tile_pool(name="msgs", bufs=MSGS_BUFS) as msgsp,
                tc.tile_pool(name="seq", bufs=4) as seqp,
                tc.tile_pool(name="sel", bufs=26) as selp,
                tc.tile_pool(name="comb", bufs=4) as combp,
                tc.tile_pool(name="hsb", bufs=4) as hsbp,
                tc.tile_pool(name="aggps", bufs=3, space="PSUM") as aggpsp,
                tc.tile_pool(name="hps", bufs=3, space="PSUM") as hpsp,
            ):
                nc.gpsimd.load_library(mlp)

                gidx_t = constp.tile([128, tot_idx // 16], mybir.dt.int16)
                nc.sync.dma_start(out=gidx_t[:], in_=gidx[:])
                dstloc_t = constp.tile([128, tot_cols], fp16)
                nc.sync.dma_start(out=dstloc_t[:], in_=dstloc[:])
                iota_t = constp.tile([128, SBD], fp16)
                nc.sync.dma_start(out=iota_t[:], in_=iota_rep[:])
                invdeg_t = constp.tile([128, ND], fp16)
                nc.sync.dma_start(out=invdeg_t[:], in_=invdeg_rep[:])
                w_ev_t = constp.tile([128, F_HID], bf16)
                nc.sync.dma_start(out=w_ev_t[:], in_=w_ev[:])
                w_od_t = constp.tile([128, F_HID], bf16)
                nc.sync.dma_start(out=w_od_t[:], in_=w_od[:])
                if has_bias:
                    bias_t = constp.tile([128, 2 * F_HID], f32)
                    nc.sync.dma_start(out=bias_t[:], in_=bias_rep[:])

                col_off = 0
                idx_off = 0
                relu_flip = 0
                for s in range(NSB):
                    KA, KB = K[s]
                    ncols = KA + KB
                    m_t = msgsp.tile([128, max_sb_cols * EW], bf16,
                                     tag="msgs")
                    m3 = m_t[:].rearrange("p (c e) -> p c e", e=EW)
                    qn = 2 * s
                    for (xsrc, h0, hn) in ((xab_lo, 0, KA),
                                           (xab_hi, KA, KB)):
                        # two calls per half: downstream matmuls overlap the
                        # second call's descgen + transfer
                        for (c0, cn) in (((h0, (hn + 1) // 2)),
                                         ((h0 + (hn + 1) // 2, hn // 2))):
                            if cn == 0:
                                continue
                            nidx = cn * 128
                            nc.gpsimd.dma_gather(
                                out_ap=m3[:, c0:c0 + cn, :],
                                in_ap=xsrc[:],
                                idxs_ap=gidx_t[:, idx_off // 16:
                                               (idx_off + nidx) // 16],
                                num_idxs=nidx,
                                num_idxs_reg=nidx,
                                elem_size=EW,
                                single_packet=False,
                                queue_num=qn % _NQUEUES,
                            )
                            idx_off += nidx
                            qn += 1

                    comb = [combp.tile([128, 4 * SBD], bf16, tag=f"comb{par}",
                                       name=f"comb{par}_{s}")
                            for par in range(2)]
                    # x/self half of comb straight from DRAM (transposed host
                    # layout); even batch -> partitions 64:128 of comb[0],
                    # odd batch -> partitions 0:64 of comb[1]
                    for b in range(BATCH):
                        par, fc = b % 2, b // 2
                        p0 = 64 if par == 0 else 0
                        nc.sync.dma_start(
                            out=comb[par][p0:p0 + 64,
                                          fc * SBD:(fc + 1) * SBD],
                            in_=xselfT[b, :, s * SBD:(s + 1) * SBD],
                        )

                    # per-group chunk lists from shared targets
                    tg = list(targets[s][0]) + list(targets[s][1])
                    g_cols = [[ci for ci in range(ncols)
                               if tg[ci][0] <= g <= tg[ci][1]]
                              for g in range(NGL)]
                    first_col = [cols[0] for cols in g_cols]
                    last_col = [cols[-1] for cols in g_cols]

                    # S strips per (half, group): contiguous runs of chunks,
                    # fp16 equality * fp16 invdeg -> bf16, <=SRUN cols per op
                    s_ap = {}   # (g, ci) -> rhs AP [128, 128]
                    for h, (lo, hi) in ((0, (0, KA)), (1, (KA, ncols))):
                        for g in range(NGL):
                            cols = [ci for ci in g_cols[g] if lo <= ci < hi]
                            if not cols:
                                continue
                            assert cols == list(range(cols[0], cols[-1] + 1))
                            if _SBUILD == "simple":
                                for ci in cols:
                                    eq_t = seqp.tile([128, SRUN * 128], fp16,
                                                     tag="seq",
                                                     name=f"eq_{s}_{h}_{g}_{ci}")
                                    nc.vector.tensor_tensor(
                                        out=eq_t[:, 0:128],
                                        in0=iota_t[:, g * 128:(g + 1) * 128],
                                        in1=dstloc_t[:, col_off + ci:
                                                     col_off + ci + 1]
                                        .to_broadcast([128, 128]),
                                        op=mybir.AluOpType.is_equal,
                                    )
                                    s_t = selp.tile([128, SRUN * 128], bf16,
                                                    tag="sel",
                                                    name=f"s_{s}_{h}_{g}_{ci}")
                                    nc.vector.tensor_tensor(
                                        out=s_t[:, 0:128],
                                        in0=eq_t[:, 0:128],
                                        in1=invdeg_t[:, s * SBD + g * 128:
                                                     s * SBD + (g + 1) * 128],
                                        op=mybir.AluOpType.mult,
                                    )
                                    s_ap[(g, ci)] = s_t[:, 0:128]
                                continue
                            for a in range(0, len(cols), SRUN):
                                run = cols[a:a + SRUN]
                                n = len(run)
                                c0 = run[0]
                                eq_t = seqp.tile([128, SRUN * 128], fp16,
                                                 tag="seq")
                                eq3 = eq_t[:].rearrange("p (c d) -> p c d",
                                                        d=128)
                                nc.vector.tensor_tensor(
                                    out=eq3[:, 0:n, :],
                                    in0=iota_t[:, g * 128:(g + 1) * 128]
                                    .rearrange("p (o d) -> p o d", o=1)
                                    .to_broadcast([128, n, 128]),
                                    in1=dstloc_t[:, col_off + c0:
                                                 col_off + c0 + n]
                                    .rearrange("p (c o) -> p c o", o=1)
                                    .to_broadcast([128, n, 128]),
                                    op=mybir.AluOpType.is_equal,
                                )
                                s_t = selp.tile([128, SRUN * 128], bf16,
                                                tag="sel")
                                s3 = s_t[:].rearrange("p (c d) -> p c d",
                                                      d=128)
                                nc.vector.tensor_tensor(
                                    out=s3[:, 0:n, :],
                                    in0=eq3[:, 0:n, :],
                                    in1=invdeg_t[:, s * SBD + g * 128:
                                                 s * SBD + (g + 1) * 128]
                                    .rearrange("p (o d) -> p o d", o=1)
                                    .to_broadcast([128, n, 128]),
                                    op=mybir.AluOpType.mult,
                                )
                                for j, ci in enumerate(run):
                                    s_ap[(g, ci)] = s3[:, j, :]

                    # aggregation matmuls, PSUM bank per group. start=True
                    # clears the has-written bits of the WHOLE bank (and
                    # accumulate-mode overwrites elements with cleared bits),
                    # so exactly ONE start per bank; everything else
                    # accumulates.
                    agg = {}
                    for ci in range(ncols):
                        gmin, gmax = tg[ci]
                        for g in range(gmin, gmax + 1):
                            fresh = g not in agg
                            if fresh:
                                agg[g] = aggpsp.tile([128, 512], f32,
                                                     tag="agg",
                                                     name=f"agg_{s}_{g}")
                            for fc in range(4):
                                nc.tensor.matmul(
                                    out=agg[g][:, fc * 128:(fc + 1) * 128],
                                    lhsT=m3[:, ci, fc * 128:(fc + 1) * 128],
                                    rhs=s_ap[(g, ci)],
                                    start=(fresh and fc == 0),
                                    stop=(ci == last_col[g] and fc == 3),
                                    skip_group_check=True,
                                )
                        # groups completed by this chunk -> copy into comb
                        for g in range(gmin, gmax + 1):
                            if last_col[g] != ci:
                                continue
                            a4 = agg[g][:].rearrange("p (f d) -> p f d", f=4)
                            c4 = [comb[par][:].rearrange("p (f d) -> p f d",
                                                         f=4)
                                  for par in range(2)]
                            dsl = slice(g * 128, (g + 1) * 128)
                            # even batches: agg parts 0:64 of comb[0]
                            nc.scalar.activation(
                                out=c4[0][0:64, :, dsl],
                                in_=a4[0:64, :, :],
                                func=mybir.ActivationFunctionType.Copy)
                            # odd batches: agg parts 64:128 of comb[1]
                            nc.scalar.activation(
                                out=c4[1][64:128, :, dsl],
                                in_=a4[64:128, :, :],
                                func=mybir.ActivationFunctionType.Copy)

                    # phase B: h = [agg;x] @ [W_l;W_r], relu, bf16 out
                    for b in range(BATCH):
                        par, fc = b % 2, b // 2
                        w_t = w_od_t if par else w_ev_t
                        for d2 in range(SBD // 256):
                            h_ps = hpsp.tile([128, 512], f32, tag="hps")
                            for j in range(2):
                                dch = d2 * 2 + j
                                nc.tensor.matmul(
                                    out=h_ps[:, j * 256:(j + 1) * 256],
                                    lhsT=comb[par][:, fc * SBD + dch * 128:
                                                   fc * SBD + (dch + 1) * 128],
                                    rhs=w_t[:],
                                    start=True,
                                    stop=True,
                                )
                            if has_bias:
                                nc.vector.tensor_add(
                                    out=h_ps[:], in0=h_ps[:], in1=bias_t[:])
                            h_t = hsbp.tile([128, 512], bf16, tag="hsb")
                            if relu_flip % 2 == 0:
                                nc.scalar.activation(
                                    out=h_t[:], in_=h_ps[:],
                                    func=mybir.ActivationFunctionType.Relu)
                            else:
                                nc.vector.tensor_relu(out=h_t[:], in_=h_ps[:])
                            relu_flip += 1
                            r0 = s * SBD + d2 * 256
                            nc.sync.dma_start(
                                out=out[b, r0:r0 + 256, :]
                                .rearrange("(k p) h -> p k h", p=128),
                                in_=h_t[:].rearrange("p (k h) -> p k h", k=2),
                            )
                    col_off += ncols
    nc.compile()
    names = dict(
        xab_lo=xab_lo.name, xab_hi=xab_hi.name, xselfT=xselfT.name,
        gidx=gidx.name, dstloc=dstloc.name, iota_rep=iota_rep.name,
        invdeg_rep=invdeg_rep.name, w_ev=w_ev.name, w_od=w_od.name,
        out=out.name, bias_rep=(bias_rep.name if has_bias else None),
    )
    return nc, names


def _prep(x, edge_src, edge_dst, W_l, b_l, W_r):
    from ml_dtypes import bfloat16

    deg = np.bincount(edge_dst, minlength=N_NODES).astype(np.float32)
    invdeg = (1.0 / np.maximum(deg, 1.0)).astype(np.float16)

    xn = np.ascontiguousarray(x.transpose(1, 0, 2)).reshape(N_NODES, EW)
    xn_bf = xn.astype(bfloat16)
    zrow = np.zeros((1, EW), dtype=bfloat16)
    xab_lo = np.ascontiguousarray(np.vstack([xn_bf[:HALF], zrow]))
    xab_hi = np.ascontiguousarray(np.vstack([xn_bf[HALF:], zrow]))

    core = edge_dst // ND
    per_core = []
    cnt = np.zeros((NCORE, NSB, 2), np.int64)
    for c in range(NCORE):
        sel = core == c
        ed = (edge_dst[sel] - c * ND).astype(np.int64)
        es = edge_src[sel].astype(np.int64)
        sb = ed // SBD
        h = (es >= HALF).astype(np.int64)
        order = np.lexsort((es, ed, h, sb))
        ed, es, sb, h = ed[order], es[order], sb[order], h[order]
        key = sb * 2 + h
        bounds = np.searchsorted(key, np.arange(2 * NSB + 1))
        cnt[c] = np.diff(bounds).reshape(NSB, 2)
        per_core.append((ed, es, bounds))

    Kmat = np.ceil(cnt.max(axis=0) / 128).astype(np.int64)
    Kmat = np.maximum(Kmat, 1)
    K = tuple((int(Kmat[s, 0]), int(Kmat[s, 1])) for s in range(NSB))

    # shared chunk -> group-range targets (union over cores)
    gmin = np.full((NSB, 2, int(Kmat.max())), NGL, np.int64)
    gmax = np.full((NSB, 2, int(Kmat.max())), -1, np.int64)
    for c in range(NCORE):
        ed, es, bounds = per_core[c]
        for s in range(NSB):
            for h in range(2):
                lo, hi = bounds[2 * s + h], bounds[2 * s + h + 1]
                n = hi - lo
                gl = (ed[lo:hi] - s * SBD) // GSZ
                for k in range(int(Kmat[s, h])):
                    a, b = k * 128, min((k + 1) * 128, n)
                    if a >= n:
                        break
                    gmin[s, h, k] = min(gmin[s, h, k], gl[a])
                    gmax[s, h, k] = max(gmax[s, h, k], gl[b - 1])
    # chunks with no real edges on any core: harmless all-zero target
    for s in range(NSB):
        for h in range(2):
            for k in range(int(Kmat[s, h])):
                if gmax[s, h, k] < 0:
                    gmin[s, h, k] = 0
                    gmax[s, h, k] = 0
        # ensure every group is targeted at least once (start=True zeroing)
        covered = np.zeros(NGL, bool)
        for h in range(2):
            for k in range(int(Kmat[s, h])):
                covered[gmin[s, h, k]:gmax[s, h, k] + 1] = True
        for g in range(NGL):
            if not covered[g]:
                h = 0 if Kmat[s, 0] > 0 else 1
                gmin[s, h, 0] = min(gmin[s, h, 0], g)
                gmax[s, h, 0] = max(gmax[s, h, 0], g)
                covered[gmin[s, h, 0]:gmax[s, h, 0] + 1] = True
    targets = tuple(
        tuple(tuple((int(gmin[s, h, k]), int(gmax[s, h, k]))
                    for k in range(int(Kmat[s, h])))
              for h in range(2))
        for s in range(NSB))
    schedule = (K, targets)

    iota_rep = np.broadcast_to(
        np.arange(SBD, dtype=np.float16)[None, :], (128, SBD)).copy()

    WlT = W_l.T.astype(np.float32)
    WrT = W_r.T.astype(np.float32)
    w_ev = np.vstack([WlT, WrT]).astype(bfloat16)
    w_od = np.vstack([WrT, WlT]).astype(bfloat16)
    has_bias = bool(np.any(b_l != 0))
    bias_rep = (np.broadcast_to(
        np.tile(b_l.astype(np.float32), 2)[None, :],
        (128, 2 * F_HID)).copy() if has_bias else None)

    in_maps = []
    for c in range(NCORE):
        ed, es, bounds = per_core[c]
        gidx_chunks = []
        dl_cols = []
        for s in range(NSB):
            for h in range(2):
                kk = int(Kmat[s, h])
                if kk == 0:
                    continue
                lo, hi = bounds[2 * s + h], bounds[2 * s + h + 1]
                n = hi - lo
                pad = HALF if (s < MSGS_BUFS or not _NEGPAD) else -1
                buf = np.full(kk * 128, pad, np.int16)
                buf[:n] = (es[lo:hi] - h * HALF).astype(np.int16)
                dbuf = np.full(kk * 128, -1.0, np.float16)
                dbuf[:n] = (ed[lo:hi] - s * SBD).astype(np.float16)
                gidx_chunks.append(np.tile(buf.reshape(-1, 16).T, (8, 1)))
                dl_cols.append(dbuf.reshape(kk, 128))
        gidx_arr = np.ascontiguousarray(
            np.concatenate(gidx_chunks, axis=1))
        dl = np.concatenate(dl_cols, axis=0)            # [tot_cols, 128]
        dstloc_arr = np.ascontiguousarray(dl.T)          # [128, tot_cols]

        xselfT_c = np.ascontiguousarray(
            xn_bf[c * ND:(c + 1) * ND].T).reshape(BATCH, F_IN, ND)
        invdeg_c = np.broadcast_to(
            invdeg[c * ND:(c + 1) * ND][None, :], (128, ND)).copy()

        in_maps.append(dict(
            xab_lo=xab_lo, xab_hi=xab_hi, xselfT=xselfT_c,
            gidx=gidx_arr, dstloc=dstloc_arr, iota_rep=iota_rep,
            invdeg_rep=invdeg_c, w_ev=w_ev, w_od=w_od, bias_rep=bias_rep,
        ))
    return schedule, has_bias, in_maps


def kernel(x, edge_src, edge_dst, W_l, b_l, W_r):
    from concourse.bass_utils import run_bass_kernel_spmd

    x = np.asarray(x, dtype=np.float32)
    edge_src = np.asarray(edge_src, dtype=np.int32)
    edge_dst = np.asarray(edge_dst, dtype=np.int32)
    W_l = np.asarray(W_l, dtype=np.float32)
    b_l = np.asarray(b_l, dtype=np.float32)
    W_r = np.asarray(W_r, dtype=np.float32)

    schedule, has_bias, in_maps = _prep(x, edge_src, edge_dst, W_l, b_l, W_r)
    key = (schedule, has_bias)
    if key not in _cache:
        _cache[key] = _build(schedule, has_bias)
    nc, names = _cache[key]

    run_maps = []
    for m in in_maps:
        rm = {names[k]: v for k, v in m.items()
              if names.get(k) is not None and v is not None}
        run_maps.append(rm)
    res = run_bass_kernel_spmd(nc, run_maps, list(range(NCORE)))
    outs = [np.asarray(res.results[c][names["out"]]) for c in range(NCORE)]
    return np.concatenate(outs, axis=1).astype(np.float32)
